# revision 1
# baseline (speedup 1.0000x reference)
"""MixHopConv (3 GIN hop-chains, N=50000, D=64, E=800000) on 8 TRN2 NeuronCores.

Self-contained Bass kernel: kernel(**inputs) takes the full (unsharded)
inputs and returns the full [50000, 64] float32 output.

Strategy (nodes sharded contiguously across 8 cores, ~6250 each):
  The binding resources are (a) SWDGE descriptor generation on the Pool
  engine (~3ns/gathered row, ~111k rows/core/round, 3 rounds) and (b) the
  random-256B HBM drain rate of the gather descriptors (~0.27 rows/ns,
  and only when concurrent drains mix both table regions). The kernel
  runs one continuous per-edge gather stream and hides the S-matrix build
  (DVE), staircase aggregation matmuls (PE), MLPs, output projection,
  shard shipping and inter-round table AllGathers underneath it.

  Per round, region-A gathers are emitted SKEW chunks ahead of region-B
  gathers (skewed interleave): the steady state alternates A/B drains
  (full HBM throughput), each round starts with an A-only run that covers
  the region-B table AllGather still in flight, and chunk MLPs + ships
  trail the B gathers so each region's next-round AllGather fires as its
  inputs complete. A-half aggregates spill to a resident zbuf (in-place
  over x^T for round 1); the chunk closes (z = zbuf + aggB, MLPs, ship)
  at its B gather. Round 1's tables are the host-replicated x (plain
  inputs), so gathering starts at t~0; the output accumulates into a
  resident SBUF tile across rounds (bo folded in at round 1).

  round 1: z1 = x + agg(x); MLP0(z1)->out_acc (SBUF-resident, bo folded),
           MLP1(z1)->a, MLP2(z1)->b; hn1 = [a|b] shipped/AllGathered.
  round 2: one gather serves both chains ([a|b] rows), dual aggregate;
           MLP1->h2 (out_acc += Wo1^T h2), MLP2->c -> hn2 ship/AllGather.
  round 3: gather c, aggregate, MLP2->h3; out = (out_acc + Wo2^T h3)^T.

Tables are [n, 128] bf16 (256B rows = dma_gather element) split into two
DRAM regions (A/B) so int16 gather indices stay in range. Edge gathers run
on all 4 SWDGE queues; queue assignment is realigned post-schedule to
Tile's DMASW lanes.
"""
import sys
import contextlib
import ctypes
import types

import numpy as np
import ml_dtypes

for _p in ("/opt/trn_rl_repo", "/opt/pypackages"):
    if _p not in sys.path:
        sys.path.append(_p)

import concourse.bass as bass
import concourse.bass_isa as bass_isa
import concourse.mybir as mybir
import concourse.tile as tile
import concourse.bacc as bacc
from concourse.bass import AP
from concourse.masks import make_identity
from concourse.library_config import mlp as mlp_lib

N_NODES = 50000
N_EDGES = 800000
N_CORES = 8
BF16 = mybir.dt.bfloat16
F32 = mybir.dt.float32
I16 = mybir.dt.int16

D = 64
CHUNK = 512
WIN = 128


def make_config(n_nodes, n_edges, n_cores=8):
    assert n_nodes % n_cores == 0
    npc = n_nodes // n_cores
    chunks = []  # (cbase, [(sbase, w), ...])
    off = 0
    while off < npc:
        cw = min(CHUNK, npc - off)
        slots = []
        soff = 0
        while soff < cw:
            w = min(WIN, cw - soff)
            slots.append((soff, w))
            soff += w
        chunks.append((off, slots))
        off += cw
    # region A = first 5 chunks: the A-table AllGather is the binding
    # collective at round boundaries, so keep it small and early. ca >= 5
    # is required: smaller A makes region B exceed 32767 rows across the 8
    # cores and the int16 gather indices overflow (silent wrong answers).
    ca = 5
    splitA = sum(sum(w for (_, w) in sl) for (_, sl) in chunks[:ca])
    return dict(n_nodes=n_nodes, n_edges=n_edges, n_cores=n_cores, npc=npc,
                chunks=chunks, chunksA=ca, splitA=splitA)


def preprocess(cfg, edge_index):
    """Bucket/sort/pad edges; build per-core idx + dstrel arrays.

    Bucket order is h-major (all region-A buckets for every chunk first,
    then all region-B buckets) to support the phase-split rounds.

    Returns dict with:
      NB[h][c][s]: uniform block counts; NI[h][c]: idxs per gather group
      per-core 'idx' [128, TOT//16] int16 (wrapped+replicated)
      per-core 'dstrel' [128, NBTOT] bf16 (pad = -1)
    """
    n_cores, npc = cfg["n_cores"], cfg["npc"]
    chunks = cfg["chunks"]
    splitA = cfg["splitA"]          # nodes per core in region A
    src = np.asarray(edge_index[0], dtype=np.int64)
    dst = np.asarray(edge_index[1], dtype=np.int64)

    # bucket[core][h][c][s] -> (srcs, dstrels)
    buckets = [[[[None for _ in ch[1]] for ch in chunks] for _ in range(2)]
               for _ in range(n_cores)]
    core_of = dst // npc
    ldst = dst - core_of * npc
    cid = np.minimum(ldst // CHUNK, len(chunks) - 1)
    src_r = src // npc
    src_j = src - src_r * npc
    hid = (src_j >= splitA).astype(np.int64)
    splitB = npc - splitA
    tpos = np.where(hid == 0, src_r * splitA + src_j,
                    src_r * splitB + (src_j - splitA))
    for r in range(n_cores):
        m_r = core_of == r
        for c, (cbase, slots) in enumerate(chunks):
            m_rc = m_r & (cid == c)
            for s, (sbase, w) in enumerate(slots):
                m = m_rc & (ldst - cbase >= sbase) & (ldst - cbase < sbase + w)
                for h in range(2):
                    mh = m & (hid == h)
                    buckets[r][h][c][s] = (tpos[mh],
                                           (ldst[mh] - cbase - sbase))

    NB = [[], []]
    for h in range(2):
        for c, (cbase, slots) in enumerate(chunks):
            NB_c = []
            for s in range(len(slots)):
                mx = max(len(buckets[r][h][c][s][0]) for r in range(n_cores))
                NB_c.append(max(1, -(-mx // 128)))
            NB[h].append(NB_c)
    NI = [[128 * sum(NB[h][c]) for c in range(len(chunks))] for h in range(2)]
    TOT = sum(NI[h][c] for h in range(2) for c in range(len(chunks)))
    NBTOT = sum(NB[h][c][s] for h in range(2) for c in range(len(chunks))
                for s in range(len(chunks[c][1])))

    per_core = []
    for r in range(n_cores):
        idx_lin = np.zeros(TOT, dtype=np.int16)
        dr_lin = np.full((128, NBTOT), -1.0, dtype=np.float32)
        ioff = 0
        boff = 0
        for h in range(2):
            for c in range(len(chunks)):
                for s in range(len(chunks[c][1])):
                    srcs, drs = buckets[r][h][c][s]
                    nb = NB[h][c][s]
                    n = len(srcs)
                    idx_lin[ioff:ioff + n] = srcs.astype(np.int16)
                    for b in range(nb):
                        lo, hi = b * 128, min((b + 1) * 128, n)
                        if hi > lo:
                            dr_lin[0:hi - lo, boff + b] = drs[lo:hi]
                    ioff += nb * 128
                    boff += nb
        assert ioff == TOT and boff == NBTOT
        # wrap: position j -> [j%16, j//16], replicate to 128 partitions
        wrapped = idx_lin.reshape(TOT // 16, 16).T  # [16, TOT//16]
        idx_arr = np.tile(wrapped, (8, 1))
        per_core.append(dict(idx=np.ascontiguousarray(idx_arr),
                             dstrel=dr_lin.astype(ml_dtypes.bfloat16)))
    return dict(NB=NB, NI=NI, TOT=TOT, NBTOT=NBTOT, per_core=per_core)


def _bcast_mid(ap, n):
    """[P, W] AP -> [P, n, W] with middle dim broadcast."""
    return AP(ap.tensor, ap.offset, [ap.ap[0], [0, n], ap.ap[1]])


def build(cfg, pp):
    """Build the 8-core SPMD program. Returns nc."""
    n_nodes, n_cores, npc = cfg["n_nodes"], cfg["n_cores"], cfg["npc"]
    chunks = cfg["chunks"]
    ca, splitA = cfg["chunksA"], cfg["splitA"]
    splitB = npc - splitA
    nA, nB = n_cores * splitA, n_cores * splitB
    NB, NI, TOT, NBTOT = pp["NB"], pp["NI"], pp["TOT"], pp["NBTOT"]
    RG = [list(range(n_cores))]
    nchunks = len(chunks)

    nc = bacc.Bacc("TRN2", target_bir_lowering=False, num_swdge_queues=4)

    # ---- parameters ----
    xtabA_in = nc.declare_dram_parameter("xtabA", [nA, 128], BF16, isOutput=False)
    xtabB_in = nc.declare_dram_parameter("xtabB", [nB, 128], BF16, isOutput=False)
    xt_in = nc.declare_dram_parameter("xt", [128, npc], BF16, isOutput=False)
    idx_in = nc.declare_dram_parameter("idx", [128, TOT // 16], I16, isOutput=False)
    dr_in = nc.declare_dram_parameter("dstrel", [128, NBTOT], BF16, isOutput=False)
    w_in = {}
    for i in range(3):
        w_in[f"w1_{i}"] = nc.declare_dram_parameter(f"w1_{i}", [D, D], BF16, isOutput=False)
        w_in[f"w2_{i}"] = nc.declare_dram_parameter(f"w2_{i}", [D, D], BF16, isOutput=False)
        w_in[f"b1_{i}"] = nc.declare_dram_parameter(f"b1_{i}", [D, 1], F32, isOutput=False)
        w_in[f"b2_{i}"] = nc.declare_dram_parameter(f"b2_{i}", [D, 1], F32, isOutput=False)
    wo_in = nc.declare_dram_parameter("wo", [3, D, D], BF16, isOutput=False)
    bo_in = nc.declare_dram_parameter("bo", [D, 1], F32, isOutput=False)
    out_ext = nc.declare_dram_parameter("out", [npc, D], F32, isOutput=True)

    # ---- internal DRAM (tables for rounds 2 and 3) ----
    bounceA = [nc.dram_tensor(f"bounceA{k}", [splitA, 128], BF16) for k in range(2)]
    bounceB = [nc.dram_tensor(f"bounceB{k}", [splitB, 128], BF16) for k in range(2)]
    tablesA = [nc.dram_tensor(f"tableA{k}", [nA, 128], BF16,
                              addr_space="Shared") for k in range(2)]
    tablesB = [nc.dram_tensor(f"tableB{k}", [nB, 128], BF16,
                              addr_space="Shared") for k in range(2)]

    NBGMAX = max(NI[h][c] // 128 for h in range(2) for c in range(nchunks))

    with tile.TileContext(nc) as tc:
        nc.gpsimd.load_library(mlp_lib)
        with (
            tc.tile_pool(name="const", bufs=1) as constp,
            tc.tile_pool(name="resident", bufs=1) as resp,
            tc.tile_pool(name="gatherA", bufs=4) as gpoolA,
            tc.tile_pool(name="gatherB", bufs=4) as gpoolB,
            tc.tile_pool(name="smatA", bufs=2) as spoolA,
            tc.tile_pool(name="smatB", bufs=2) as spoolB,
            tc.tile_pool(name="strip", bufs=2) as stripp,
            tc.tile_pool(name="psaggA", bufs=1, space="PSUM") as psaggA,
            tc.tile_pool(name="psaggB", bufs=2, space="PSUM") as psaggB,
            tc.tile_pool(name="psmlp", bufs=2, space="PSUM") as psmlp,
            tc.tile_pool(name="pstp", bufs=1, space="PSUM") as pstp,
        ):
            # ---- constants / resident ----
            iota16 = constp.tile([128, WIN], I16)
            nc.gpsimd.iota(iota16[:], pattern=[[1, WIN]], base=0, channel_multiplier=0)
            iota_b = constp.tile([128, WIN], BF16)
            nc.vector.tensor_copy(iota_b[:], iota16[:])
            iota_rep = constp.tile([128, NBGMAX * WIN], BF16)
            rep = 1
            nc.vector.tensor_copy(iota_rep[:, 0:WIN], iota_b[:])
            while rep < NBGMAX:
                n = min(rep, NBGMAX - rep)
                nc.vector.tensor_copy(iota_rep[:, rep * WIN:(rep + n) * WIN],
                                      iota_rep[:, 0:n * WIN])
                rep += n
            id128 = constp.tile([128, 128], BF16)
            make_identity(nc, id128[:])
            id64 = constp.tile([D, D], F32)
            make_identity(nc, id64[:])

            idx_t = resp.tile([128, TOT // 16], I16)
            nc.sync.dma_start(idx_t[:], idx_in[:])
            dr_t = resp.tile([128, NBTOT], BF16)
            nc.sync.dma_start(dr_t[:], dr_in[:])
            zbuf = resp.tile([128, npc], BF16, tag="zbuf", name="zbuf")
            nc.sync.dma_start(zbuf[:], xt_in[:])

            wt = {}
            for i in range(3):
                for nm in (f"w1_{i}", f"w2_{i}"):
                    wt[nm] = constp.tile([D, D], BF16, tag=nm, name=nm)
                    nc.sync.dma_start(wt[nm][:], w_in[nm][:])
                for nm in (f"b1_{i}", f"b2_{i}"):
                    wt[nm] = constp.tile([D, 1], F32, tag=nm, name=nm)
                    nc.sync.dma_start(wt[nm][:], w_in[nm][:])
            wo_t = [constp.tile([D, D], BF16, tag=f"wo{k}", name=f"wo{k}") for k in range(3)]
            for k in range(3):
                nc.sync.dma_start(wo_t[k][:], wo_in[k])
            bo_t = constp.tile([D, 1], F32)
            nc.sync.dma_start(bo_t[:], bo_in[:])

            hn1 = resp.tile([128, npc], BF16, tag="hn1", name="hn1")
            hn2 = resp.tile([128, npc], BF16, tag="hn2", name="hn2")
            out_acc = resp.tile([D, npc], BF16, tag="oacc", name="out_acc")

            # linear offsets of the A- and B-half bucket groups
            ioffA = [0] * nchunks
            boffA = [0] * nchunks
            io = 0
            bo_ = 0
            for c in range(nchunks):
                ioffA[c] = io
                boffA[c] = bo_
                io += NI[0][c]
                bo_ += NI[0][c] // 128
            ioffB = [0] * nchunks
            boffB = [0] * nchunks
            for c in range(nchunks):
                ioffB[c] = io
                boffB[c] = bo_
                io += NI[1][c]
                bo_ += NI[1][c] // 128
            assert io == TOT and bo_ == NBTOT

            gq = [0]

            def gather_half(h, c, tab, pool, spool_):
                """Issue gather + S-build for (half h, chunk c). Returns (g, S, nbg)."""
                ni = NI[h][c]
                nbg = ni // 128
                ioff = (ioffA if h == 0 else ioffB)[c]
                boff = (boffA if h == 0 else boffB)[c]
                g = pool.tile([128, nbg, 128], BF16, tag=f"g{h}", name=f"g{h}")
                nc.gpsimd.dma_gather(
                    g[:], tab[:], idx_t[:, ioff // 16:(ioff + ni) // 16],
                    ni, ni, 128, elem_step=128, single_packet=False,
                    queue_num=gq[0] % 4)
                gq[0] += 1
                S = spool_.tile([128, nbg * WIN], BF16, tag=f"S{h}", name=f"S{h}")
                slots = chunks[c][1]
                wmax = max(w for (_, w) in slots)
                if wmax == WIN:
                    nc.vector.tensor_tensor(
                        out=S[:],
                        in0=AP(dr_t[:].tensor,
                               dr_t[:, boff:boff + nbg].offset,
                               [dr_t[:].ap[0], [1, nbg], [0, WIN]]),
                        in1=iota_rep[:, 0:nbg * WIN],
                        op=mybir.AluOpType.is_equal)
                else:
                    nc.vector.tensor_tensor(
                        out=AP(S[:].tensor, S[:].offset,
                               [S[:].ap[0], [WIN, nbg], [1, wmax]]),
                        in0=dr_t[:, boff:boff + nbg].to_broadcast([128, nbg, wmax]),
                        in1=_bcast_mid(iota_b[:, :wmax], nbg),
                        op=mybir.AluOpType.is_equal)
                return g, S

            def staircase(h, c, g, S, ps):
                """Accumulate half-h blocks of chunk c into psum tile ps."""
                slots = chunks[c][1]
                bi = 0
                for s, (sbase, w) in enumerate(slots):
                    nb = NB[h][c][s]
                    for b in range(nb):
                        nc.tensor.matmul(
                            ps[:, sbase:sbase + w],
                            lhsT=g[:, bi, :],
                            rhs=S[:, bi * WIN:bi * WIN + w],
                            start=(b == 0),
                            stop=(b == nb - 1))
                        bi += 1

            def mlp_strip(z_ap, i, w):
                """relu(z@W1+b1)@W2 in transposed layout; returns [64,w] f32 psum."""
                p1 = psmlp.tile([D, CHUNK], F32, tag="pm", name="p1")
                nc.tensor.matmul(p1[:, :w], lhsT=wt[f"w1_{i}"][:], rhs=z_ap,
                                 start=True, stop=True)
                m = stripp.tile([D, CHUNK], BF16, tag="m", name="m", bufs=3)
                nc.scalar.activation(m[:, :w], p1[:, :w],
                                     mybir.ActivationFunctionType.Relu,
                                     bias=wt[f"b1_{i}"][:])
                p2 = psmlp.tile([D, CHUNK], F32, tag="p2", name="p2", bufs=1)
                nc.tensor.matmul(p2[:, :w], lhsT=wt[f"w2_{i}"][:], rhs=m[:, :w],
                                 start=True, stop=True)
                return p2

            def proj_out(p2, i, k, w):
                """h = p2 + b2 (scalar); return po = Wo_k^T h psum tile."""
                hstrip = stripp.tile([D, CHUNK], BF16, tag="h", name="h", bufs=3)
                nc.scalar.activation(hstrip[:, :w], p2[:, :w],
                                     mybir.ActivationFunctionType.Identity,
                                     bias=wt[f"b2_{i}"][:])
                po = psmlp.tile([D, CHUNK], F32, tag="pm", name="po")
                nc.tensor.matmul(po[:, :w], lhsT=wo_t[k][:], rhs=hstrip[:, :w],
                                 start=True, stop=True)
                return po

            def ship_tiles(hsrc, dstten, rowbase, colbase, width):
                """transpose hsrc[:, colbase:+width] into dstten[rowbase:]."""
                t0 = 0
                while t0 < width:
                    tw = min(128, width - t0)
                    pt = pstp.tile([128, 128], BF16, tag="tp", name="tp")
                    nc.tensor.transpose(pt[:tw, :],
                                        hsrc[:, colbase + t0:colbase + t0 + tw],
                                        id128[:])
                    st = stripp.tile([128, 128], BF16, tag="ship", name="ship",
                                     bufs=10)
                    nc.scalar.activation(st[:tw, :], pt[:tw, :],
                                         mybir.ActivationFunctionType.Copy)
                    nc.sync.dma_start(dstten[rowbase + t0:rowbase + t0 + tw, :],
                                      st[:tw, :])
                    t0 += tw

            SKEW = 6   # A-gathers run this many chunks ahead of B-gathers

            for rk in range(3):  # rounds 1..3
                tabA = xtabA_in if rk == 0 else tablesA[rk - 1]
                tabB = xtabB_in if rk == 0 else tablesB[rk - 1]
                h_own = (zbuf, hn1, hn2)[rk]   # rk 0: in-place x + agg
                P = 128 if rk == 1 else D      # round 2 carries two chains

                def do_A(c):
                    cbase, slots = chunks[c]
                    cw = sum(w for (_, w) in slots)
                    g, S = gather_half(0, c, tabA, gpoolA, spoolA)
                    psA = psaggA.tile([128, CHUNK], F32, tag="aggA", name="aggA")
                    staircase(0, c, g, S, psA)
                    with tc.high_priority():
                        nc.vector.tensor_tensor(
                            zbuf[0:P, cbase:cbase + cw], psA[0:P, :cw],
                            h_own[0:P, cbase:cbase + cw],
                            op=mybir.AluOpType.add)
                    if rk == 0 and c == 2:
                        # zero hn2's upper half (becomes round-3 table cols
                        # 64..127) off the Pool-engine ramp
                        nc.gpsimd.memset(hn2[D:128, :], 0.0)

                def do_B(c):
                    cbase, slots = chunks[c]
                    cw = sum(w for (_, w) in slots)
                    g, S = gather_half(1, c, tabB, gpoolB, spoolB)
                    psB = psaggB.tile([128, CHUNK], F32, tag="aggB", name="aggB")
                    staircase(1, c, g, S, psB)
                    hp = tc.high_priority()
                    hp.__enter__()
                    z1 = stripp.tile([D, CHUNK], BF16, tag="z1", name="z1", bufs=3)
                    nc.vector.tensor_tensor(
                        z1[:, :cw], psB[0:D, :cw], zbuf[0:D, cbase:cbase + cw],
                        op=mybir.AluOpType.add)
                    if rk == 0:
                        p2 = mlp_strip(z1[:, :cw], 0, cw)
                        po = proj_out(p2, 0, 0, cw)
                        nc.scalar.activation(out_acc[:, cbase:cbase + cw],
                                             po[:, :cw],
                                             mybir.ActivationFunctionType.Identity,
                                             bias=bo_t[:])
                        p2 = mlp_strip(z1[:, :cw], 1, cw)
                        nc.scalar.activation(hn1[0:D, cbase:cbase + cw],
                                             p2[:, :cw],
                                             mybir.ActivationFunctionType.Identity,
                                             bias=wt["b2_1"][:])
                        p2 = mlp_strip(z1[:, :cw], 2, cw)
                        nc.vector.tensor_tensor(
                            hn1[D:128, cbase:cbase + cw], p2[:, :cw],
                            wt["b2_2"][:].to_broadcast([D, cw]),
                            op=mybir.AluOpType.add)
                    elif rk == 1:
                        z2 = stripp.tile([D, CHUNK], BF16, tag="z2", name="z2")
                        nc.vector.tensor_tensor(
                            z2[:, :cw], psB[D:128, :cw],
                            zbuf[D:128, cbase:cbase + cw],
                            op=mybir.AluOpType.add)
                        p2 = mlp_strip(z1[:, :cw], 1, cw)
                        po = proj_out(p2, 1, 1, cw)
                        nc.vector.tensor_tensor(
                            out_acc[:, cbase:cbase + cw],
                            out_acc[:, cbase:cbase + cw], po[:, :cw],
                            op=mybir.AluOpType.add)
                        p2 = mlp_strip(z2[:, :cw], 2, cw)
                        nc.scalar.activation(hn2[0:D, cbase:cbase + cw],
                                             p2[:, :cw],
                                             mybir.ActivationFunctionType.Identity,
                                             bias=wt["b2_2"][:])
                    else:
                        p2 = mlp_strip(z1[:, :cw], 2, cw)
                        po = proj_out(p2, 2, 2, cw)
                        fs = stripp.tile([D, CHUNK], F32, tag="fs", name="fs")
                        nc.vector.tensor_tensor(
                            fs[:, :cw], out_acc[:, cbase:cbase + cw],
                            po[:, :cw], op=mybir.AluOpType.add)
                        t0 = 0
                        while t0 < cw:
                            tw = min(128, cw - t0)
                            pt = pstp.tile([128, D], F32, tag="ftp", name="ftp")
                            nc.tensor.transpose(pt[:tw, :], fs[:, t0:t0 + tw],
                                                id64[:])
                            os = stripp.tile([128, D], F32, tag="fout",
                                             name="fout", bufs=4)
                            nc.scalar.activation(
                                os[:tw, :], pt[:tw, :],
                                mybir.ActivationFunctionType.Copy)
                            nc.sync.dma_start(
                                out_ext[cbase + t0:cbase + t0 + tw, :],
                                os[:tw, :])
                            t0 += tw
                    # ship this chunk's next-round table shard
                    if rk < 2:
                        hsrc = hn1 if rk == 0 else hn2
                        bA, bB = bounceA[rk], bounceB[rk]
                        if cbase + cw <= splitA:
                            ship_tiles(hsrc, bA, cbase, cbase, cw)
                        else:
                            ship_tiles(hsrc, bB, cbase - splitA, cbase, cw)
                    hp.__exit__(None, None, None)
                    if rk < 2 and c == ca - 1:
                        # region-A table AllGather: chunks 0..6 shipped
                        nc.gpsimd.collective_compute(
                            "AllGather", mybir.AluOpType.bypass,
                            replica_groups=RG, ins=[bounceA[rk][:]],
                            outs=[tablesA[rk][:]])
                    if rk < 2 and c == nchunks - 1:
                        nc.gpsimd.collective_compute(
                            "AllGather", mybir.AluOpType.bypass,
                            replica_groups=RG, ins=[bounceB[rk][:]],
                            outs=[tablesB[rk][:]])

                # skewed interleave: A runs `sk` chunks ahead of B so round
                # boundaries have A-only work while table-B's AG lands, and
                # steady state mixes A/B drains (higher HBM throughput).
                # Round 1 has no AG dependency (tables are inputs), so it
                # interleaves fully from t=0 instead of an A-only prologue.
                sk = (1, SKEW, SKEW + 2)[rk]
                for c in range(sk):
                    do_A(c)
                for c in range(sk, nchunks):
                    do_B(c - sk)
                    do_A(c)
                for c in range(nchunks - sk, nchunks):
                    do_B(c)

    # Align each Pool-engine DMA's SWDGE queue with Tile's DMASW lane
    # rotation (lane = i % 8 over scheduled Pool DMA order; ucode requires a
    # lane's completion sem to be driven by a single queue).
    pool_dma_i = 0
    for f in nc.m.functions:
        for blk in f.blocks:
            for inst in blk.instructions:
                if (inst.engine == mybir.EngineType.Pool
                        and isinstance(inst, bass_isa.AnyDMAInstruction)
                        and not isinstance(inst, mybir.InstCollectiveCompute)):
                    if hasattr(inst, "queue_num"):
                        inst.queue_num = (pool_dma_i % 8) % 4
                    pool_dma_i += 1
    nc.compile()
    return nc


def host_inputs(cfg, pp, x, weights):
    """Build per-core in_maps. x: [n_nodes, 64] f32. weights: dict of reference arrays."""
    n_cores, npc = cfg["n_cores"], cfg["npc"]
    splitA = cfg["splitA"]
    bf = ml_dtypes.bfloat16
    in_maps = []
    wo = np.asarray(weights["Wo"], dtype=np.float32).reshape(3, D, D).astype(bf)
    bo = np.asarray(weights["bo"], dtype=np.float32).reshape(D, 1)

    # replicated round-1 tables: region-A rows then region-B rows, rank-major
    xf = np.asarray(x, dtype=np.float32)
    xpad = np.zeros((cfg["n_nodes"], 128), dtype=bf)
    xpad[:, :D] = xf.astype(bf)
    xs3 = xpad.reshape(n_cores, npc, 128)
    xtabA = np.ascontiguousarray(xs3[:, :splitA].reshape(-1, 128))
    xtabB = np.ascontiguousarray(xs3[:, splitA:].reshape(-1, 128))

    for r in range(n_cores):
        m = {}
        xs = xf[r * npc:(r + 1) * npc]
        m["xtabA"] = xtabA
        m["xtabB"] = xtabB
        xt = np.zeros((128, npc), dtype=bf)
        xt[:D, :] = xs.T.astype(bf)
        m["xt"] = xt
        m["idx"] = pp["per_core"][r]["idx"]
        m["dstrel"] = pp["per_core"][r]["dstrel"]
        for i in range(3):
            m[f"w1_{i}"] = np.asarray(weights[f"W1_{i}"], np.float32).astype(bf)
            m[f"w2_{i}"] = np.asarray(weights[f"W2_{i}"], np.float32).astype(bf)
            m[f"b1_{i}"] = np.asarray(weights[f"b1_{i}"], np.float32).reshape(D, 1)
            m[f"b2_{i}"] = np.asarray(weights[f"b2_{i}"], np.float32).reshape(D, 1)
        m["wo"] = wo
        m["bo"] = bo
        in_maps.append(m)
    return in_maps


_PROF_SO = "/opt/axon/libaxon_pjrt.so"


def _install_profile_shim():
    """Provide antenv.axon_hooks (absent in some containers) so
    run_bass_kernel_spmd(trace=True) can capture NTFF profiles."""
    try:
        import antenv
    except ImportError:
        return
    if getattr(antenv, "axon_hooks", None) is not None:
        return

    def _hook_factory(so_path):
        try:
            lib = ctypes.CDLL(so_path)
        except OSError:
            return None
        if not hasattr(lib, "axon_start_nrt_profile"):
            return None
        lib.axon_start_nrt_profile.argtypes = [ctypes.POINTER(ctypes.c_int64),
                                               ctypes.c_size_t]
        lib.axon_start_nrt_profile.restype = ctypes.c_int64
        lib.axon_stop_nrt_profile.argtypes = [ctypes.c_char_p]
        lib.axon_stop_nrt_profile.restype = ctypes.c_int64

        @contextlib.contextmanager
        def _hook(output_dir, device_ids):
            import jax
            jax.devices()
            if device_ids:
                ids = (ctypes.c_int64 * len(device_ids))(*device_ids)
                rc = lib.axon_start_nrt_profile(ids, len(device_ids))
            else:
                rc = lib.axon_start_nrt_profile(None, 0)
            if rc != 0:
                raise RuntimeError(f"axon_start_nrt_profile rc={rc}")
            try:
                yield
            finally:
                n = lib.axon_stop_nrt_profile(str(output_dir).encode())
                print(f"profile: {n} file(s) written to {output_dir}",
                      file=sys.stderr)

        return _hook

    mod = types.ModuleType("antenv.axon_hooks")
    _state = {"hook": _hook_factory(_PROF_SO)}
    mod.set_axon_ntff_profile_hook = lambda h: _state.__setitem__("hook", h)
    mod.get_axon_ntff_profile_hook = lambda: _state["hook"]
    sys.modules["antenv.axon_hooks"] = mod
    antenv.axon_hooks = mod
    import concourse.bass_utils as _bu
    _bu.upload_artifacts = lambda tmpdir: f"local://{tmpdir}"


_CACHE = {}


def _get_program(edge_index):
    key = hash(edge_index.tobytes())
    if key not in _CACHE:
        cfg = make_config(N_NODES, N_EDGES, N_CORES)
        pp = preprocess(cfg, edge_index)
        nc = build(cfg, pp)
        _CACHE[key] = (cfg, pp, nc)
    return _CACHE[key]


def run(trace=False, **inputs):
    """Run the kernel; returns (output [N_NODES, 64] f32, exec_time_ns|None)."""
    from concourse.bass_utils import run_bass_kernel_spmd

    x = np.asarray(inputs["x"], dtype=np.float32)
    edge_index = np.asarray(inputs["edge_index"], dtype=np.int64)
    weights = {k: np.asarray(v) for k, v in inputs.items()
               if k not in ("x", "edge_index")}
    assert x.shape == (N_NODES, D) and edge_index.shape == (2, N_EDGES)

    if trace:
        _install_profile_shim()
    cfg, pp, nc = _get_program(edge_index)
    in_maps = host_inputs(cfg, pp, x, weights)
    res = run_bass_kernel_spmd(nc, in_maps, list(range(N_CORES)), trace=trace)
    out = np.concatenate([res.results[r]["out"] for r in range(N_CORES)],
                         axis=0).astype(np.float32)
    return out, res.exec_time_ns


def kernel(**inputs):
    out, _ = run(trace=False, **inputs)
    return out

